# revision 29
# baseline (speedup 1.0000x reference)
"""Trainium2 Bass kernel for DotAtten (see reference):
    qk = q[:,None,:,:] * q[:,:,None,:]; h = tanh(qk @ Wd); sjt = h @ vd
    atten = softmax(sjt, axis=2); context = atten @ value
    returns (context, atten)

Sharding: B*S = 1024 output rows -> 128 rows per core (cores 0-3 batch
0, cores 4-7 batch 1). Each core gets the full Q^T / V of its batch,
rolled along the sequence axis so its own s-block sits at [0:128] --
the SPMD program is identical across cores, all specialization lives
in the data.

v2 optimizations over the fp32r baseline:
  - sjt is symmetric in (s, t) (both factors come from query), so row j
    only computes local columns [j, 512); the missing lower-left block
    [0:j) lies entirely inside the core's own 128x128 diagonal block
    and is reconstructed at the tail via one PE transpose + two
    affine_selects + one add.
  - main matmul operands (w2, qt) and the tanh output zt are bf16:
    same PE stream rate, but 2x DVE throughput for the w2 build and
    fast weight load for the 128-col stationary tiles.
  - the vd-reduction matmuls run 4 rows concurrently in 4 PE column
    strips (tile_position=(0, 32u)), quartering their stream cost;
    score rows land on psum partitions {0,32,64,96} and are staged in
    one strided copy per 4 rows instead of one [1, S] copy per 2 rows.
"""

import os
from contextlib import ExitStack

import numpy as np

B, S, E2, E = 2, 512, 512, 256
N_CORES = 8
ROWS = (B * S) // N_CORES
P = 128
HC = E2 // P
TC = S // P
DC = 2

_STATE = {}


def _build_nc(skew=3, pz_bufs=3):
    import concourse.bacc as bacc
    import concourse.bass as bass
    import concourse.tile as tile
    from concourse import mybir
    from concourse.masks import make_identity

    f32 = mybir.dt.float32
    f32r = mybir.dt.float32r
    bf16 = mybir.dt.bfloat16

    nc = bacc.Bacc("TRN2", target_bir_lowering=False, debug=False, num_devices=8)
    qt_h = nc.dram_tensor("qt", [E2, S], bf16, kind="ExternalInput")
    v_h = nc.dram_tensor("v", [S, E2], bf16, kind="ExternalInput")
    w_h = nc.dram_tensor("w", [E2, E], bf16, kind="ExternalInput")
    vd2_h = nc.dram_tensor("vd2", [2 * E], bf16, kind="ExternalInput")
    ctx_h = nc.dram_tensor("ctx_out", [ROWS, E2], f32, kind="ExternalOutput")
    att_h = nc.dram_tensor("att_out", [ROWS, S], f32, kind="ExternalOutput")
    qt, v, w = qt_h.ap(), v_h.ap(), w_h.ap()
    ctx_out, att_out = ctx_h.ap(), att_h.ap()

    with tile.TileContext(nc) as tc, ExitStack() as ctx:
        consts = ctx.enter_context(tc.tile_pool(name="consts", bufs=1))
        w2_pool = ctx.enter_context(tc.tile_pool(name="w2", bufs=6))
        zt_pool = ctx.enter_context(tc.tile_pool(name="zt", bufs=10))
        ps_pool = ctx.enter_context(tc.tile_pool(name="ps", bufs=1, space="PSUM"))
        stage_pool = ctx.enter_context(tc.tile_pool(name="stage", bufs=2))
        tail_pool = ctx.enter_context(tc.tile_pool(name="tail", bufs=1))

        qt_bf = consts.tile([P, HC, S], bf16)
        w_bf = consts.tile([P, HC, E], bf16)
        for hc in range(HC):
            # interleave qt/w chunk loads: the first w2 build + matmuls
            # need only (qt0, w0), so they start after ~2 small DMAs
            nc.sync.dma_start(out=qt_bf[:, hc, :], in_=qt[hc*P:(hc+1)*P, :])
            nc.sync.dma_start(out=w_bf[:, hc, :], in_=w[hc*P:(hc+1)*P, :])
        vd_sb = consts.tile([P, DC, 1], bf16)
        vd_src = bass.AP(tensor=vd2_h, offset=0, ap=[[1, P], [P, DC]])
        nc.sync.dma_start(out=vd_sb[:, :, 0], in_=vd_src)
        ident = consts.tile([P, P], bf16)
        make_identity(nc, ident[:])
        ident_f = consts.tile([P, P], f32)
        make_identity(nc, ident_f[:])

        qt_bf_base = qt_bf[:]

        def emit_front(j):
            w2 = w2_pool.tile([P, HC, E], bf16, tag="w2", name=f"w2_{j}")
            if j < 4:
                # startup: per-hc build so hc-0 matmuls start after the
                # first qt chunk lands instead of all four
                for hc in range(HC):
                    qb = bass.AP(tensor=qt_bf_base.tensor,
                                 offset=qt_bf_base.offset + hc * S + j,
                                 ap=[[qt_bf_base.ap[0][0], P], [0, E]])
                    nc.vector.tensor_tensor(out=w2[:, hc, :],
                                            in0=w_bf[:, hc, :], in1=qb,
                                            op=mybir.AluOpType.mult)
            else:
                qb = bass.AP(tensor=qt_bf_base.tensor,
                             offset=qt_bf_base.offset + j,
                             ap=[[qt_bf_base.ap[0][0], P], [S, HC], [0, E]])
                nc.vector.tensor_tensor(out=w2[:], in0=w_bf[:], in1=qb,
                                        op=mybir.AluOpType.mult)
            # one 1-bank psum tile per dc chunk: tanh evacuates the dc0
            # half while dc1 is still accumulating, halving the window a
            # psum buffer stays live and the recycle stalls it causes
            zt = zt_pool.tile([P, DC, S], bf16, tag="zt", name=f"zt_{j}")
            for dc in range(DC):
                pz = ps_pool.tile([P, S], f32, tag=f"pz{dc}", bufs=pz_bufs,
                                  name=f"pz{dc}_{j}")
                for hc in range(HC):
                    nc.tensor.matmul(pz[:, j:S],
                                     lhsT=w2[:, hc, dc*P:(dc+1)*P],
                                     rhs=qt_bf[:, hc, j:S],
                                     start=(hc == 0), stop=(hc == HC-1))
                nc.scalar.activation(zt[:, dc, j:S], pz[:, j:S],
                                     mybir.ActivationFunctionType.Tanh)
            return zt

        def emit_reduce_group(j0, zts):
            # 4 rows in 4 PE column strips; scores land on psum
            # partitions {0, 32, 64, 96} of one bank
            psr = ps_pool.tile([P, S], f32, tag="psr", bufs=2,
                               name=f"psr_{j0}")
            for u in range(4):
                j = j0 + u
                zt = zts.pop(j)
                for dc in range(DC):
                    nc.tensor.matmul(psr[32*u:32*u+1, j:S],
                                     lhsT=vd_sb[:, dc, :],
                                     rhs=zt[:, dc, j:S],
                                     start=(dc == 0), stop=(dc == DC-1),
                                     tile_position=(0, 32*u))
            stage = stage_pool.tile([P, S], f32, tag="rowstage",
                                    name=f"rowstage_{j0}")
            # full-partition copy costs the same per lane as a single row;
            # rows sit at partitions {0,32,64,96}, the rest is junk.
            # On vector, not scalar: the scalar FIFO must stay free for
            # tanh, which gates pz-buffer recycling.
            nc.vector.tensor_copy(stage[:, j0:S], psr[:, j0:S])
            # partition-crossing SBUF->SBUF DMA straight into the softmax
            # tile -- no DRAM round trip. Issue from different engine
            # queues so the last group's four issues don't serialize.
            qeng = [nc.sync, nc.gpsimd, nc.scalar, nc.gpsimd]
            for u in range(4):
                j = j0 + u
                qeng[u].dma_start(out=sjt_gather[j:j+1, j:S],
                                  in_=stage[32*u:32*u+1, j:S])

        sjt_gather = tail_pool.tile([P, S], f32)
        zts = {}
        for step in range(ROWS + skew + 4):
            if step < ROWS:
                zts[step] = emit_front(step)
            r = step - skew
            if r >= 0 and r % 4 == 3 and r - 3 < ROWS:
                emit_reduce_group(r - 3, zts)
            if step == 1:
                # v only needed at the tail — load behind the first rows
                v_sb = consts.tile([P, TC, E2], bf16)
                for tcc in range(TC):
                    nc.sync.dma_start(out=v_sb[:, tcc, :],
                                      in_=v[tcc*P:(tcc+1)*P, :])

        # tail: fix the diagonal block, softmax + context
        sjt = sjt_gather
        # reconstruct lower triangle of the own-diagonal block:
        #   sjt[a, b] (a > b) = sjt[b, a]
        pT = ps_pool.tile([P, P], f32, tag="pz0", bufs=pz_bufs, name="pT")
        nc.tensor.transpose(pT[:], sjt[:, 0:P], ident_f[:])
        tT = tail_pool.tile([P, P], f32)
        nc.vector.tensor_copy(tT[:], pT[:])
        # keep strict-lower of the transpose (x > y), else 0
        nc.gpsimd.affine_select(out=tT[:], in_=tT[:],
                                compare_op=mybir.AluOpType.is_ge, fill=0.0,
                                base=-1, pattern=[[-1, P]],
                                channel_multiplier=1)
        # keep upper-incl-diag of the computed block (y >= x), else 0
        nc.gpsimd.affine_select(out=sjt[:, 0:P], in_=sjt[:, 0:P],
                                compare_op=mybir.AluOpType.is_ge, fill=0.0,
                                base=0, pattern=[[1, P]],
                                channel_multiplier=-1)
        nc.vector.tensor_tensor(out=sjt[:, 0:P], in0=sjt[:, 0:P], in1=tT[:],
                                op=mybir.AluOpType.add)

        # no max-subtraction: |sjt| <= sum|vd| ~ 13, exp is fp32-safe and
        # softmax is shift-invariant. The non-diagonal columns don't need
        # the diag fix, so their exp/cast/transposes overlap it.
        att = tail_pool.tile([P, S], f32)
        denom_a = tail_pool.tile([P, 1], f32)
        denom_b = tail_pool.tile([P, 1], f32)
        att_bf = tail_pool.tile([P, S], bf16)
        nc.scalar.activation(att[:, P:S], sjt[:, P:S],
                             mybir.ActivationFunctionType.Exp,
                             bias=0.0, scale=1.0, accum_out=denom_b[:])
        nc.vector.tensor_copy(att_bf[:, P:S], att[:, P:S])
        nc.scalar.activation(att[:, 0:P], sjt[:, 0:P],
                             mybir.ActivationFunctionType.Exp,
                             bias=0.0, scale=1.0, accum_out=denom_a[:])
        nc.vector.tensor_copy(att_bf[:, 0:P], att[:, 0:P])
        denom = tail_pool.tile([P, 1], f32)
        nc.vector.tensor_tensor(out=denom[:], in0=denom_a[:], in1=denom_b[:],
                                op=mybir.AluOpType.add)
        rdenom = tail_pool.tile([P, 1], f32)
        nc.vector.reciprocal(rdenom[:], denom[:])
        atten = tail_pool.tile([P, S], f32)
        nc.vector.tensor_scalar_mul(atten[:], att[:], rdenom[:])
        nc.sync.dma_start(out=att_out[:, :], in_=atten[:])

        # context from UNNORMALIZED exp; scale rows by 1/denom afterwards.
        # Transpose non-diag blocks first: their operand is ready earlier,
        # and psum accumulation order is free (start on the first emitted).
        attT = tail_pool.tile([P, TC, P], bf16)
        order = [1, 2, 3, 0]
        for tcc in order:
            pt2 = ps_pool.tile([P, P], bf16, tag="pz0", bufs=pz_bufs,
                               name=f"pt2_{tcc}")
            nc.tensor.transpose(pt2[:], att_bf[:, tcc*P:(tcc+1)*P], ident[:])
            nc.vector.tensor_copy(attT[:, tcc, :], pt2[:])
        pc = ps_pool.tile([P, E2], f32, tag="psr", bufs=2, name="pc")
        for k, tcc in enumerate(order):
            nc.tensor.matmul(pc[:], lhsT=attT[:, tcc, :], rhs=v_sb[:, tcc, :],
                             start=(k == 0), stop=(k == TC-1))
        ctx_sb = tail_pool.tile([P, E2], f32)
        nc.scalar.activation(ctx_sb[:], pc[:],
                             mybir.ActivationFunctionType.Identity,
                             scale=rdenom[:])
        nc.sync.dma_start(out=ctx_out[:, :], in_=ctx_sb[:])

    nc.compile()
    return nc


def _get_nc():
    if "nc" not in _STATE:
        _STATE["nc"] = _build_nc()
    return _STATE["nc"]


def kernel(query, value, Wd, vd):
    import ml_dtypes
    from concourse.bass_utils import run_bass_kernel_spmd

    bf = ml_dtypes.bfloat16
    query = np.asarray(query, dtype=np.float32).astype(bf)
    value = np.asarray(value, dtype=np.float32).astype(bf)
    Wd = np.asarray(Wd, dtype=np.float32).astype(bf)
    vd = np.asarray(vd, dtype=np.float32).astype(bf)

    vd2 = np.concatenate([vd, vd])
    in_maps = []
    for c in range(N_CORES):
        b, s0 = divmod(c * ROWS, S)
        qt = np.ascontiguousarray(np.roll(query[b].T, -s0, axis=1))
        vr = np.ascontiguousarray(np.roll(value[b], -s0, axis=0))
        in_maps.append({"qt": qt, "v": vr, "w": Wd, "vd2": vd2})

    nc = _get_nc()
    trace = bool(int(os.environ.get("BASS_KERNEL_TRACE", "0")))
    res = run_bass_kernel_spmd(nc, in_maps, list(range(N_CORES)), trace=trace)
    _STATE["last_result"] = res

    context = np.empty((B, S, E2), np.float32)
    atten = np.empty((B, S, S), np.float32)
    for c in range(N_CORES):
        b, s0 = divmod(c * ROWS, S)
        context[b, s0:s0 + ROWS] = res.results[c]["ctx_out"]
        atten[b, s0:s0 + ROWS] = np.roll(res.results[c]["att_out"], s0, axis=1)
    return context, atten


# revision 31
# speedup vs baseline: 1.0041x; 1.0041x over previous
"""Trainium2 Bass kernel for DotAtten (see reference):
    qk = q[:,None,:,:] * q[:,:,None,:]; h = tanh(qk @ Wd); sjt = h @ vd
    atten = softmax(sjt, axis=2); context = atten @ value
    returns (context, atten)

Sharding: B*S = 1024 output rows -> 128 rows per core (cores 0-3 batch
0, cores 4-7 batch 1). Each core gets the full Q^T / V of its batch,
rolled along the sequence axis so its own s-block sits at [0:128] --
the SPMD program is identical across cores, all specialization lives
in the data.

v2 optimizations over the fp32r baseline:
  - sjt is symmetric in (s, t) (both factors come from query), so row j
    only computes local columns [j, 512); the missing lower-left block
    [0:j) lies entirely inside the core's own 128x128 diagonal block
    and is reconstructed at the tail via one PE transpose + two
    affine_selects + one add.
  - main matmul operands (w2, qt) and the tanh output zt are bf16:
    same PE stream rate, but 2x DVE throughput for the w2 build and
    fast weight load for the 128-col stationary tiles.
  - the vd-reduction matmuls run 4 rows concurrently in 4 PE column
    strips (tile_position=(0, 32u)), quartering their stream cost;
    score rows land on psum partitions {0,32,64,96} and are staged in
    one strided copy per 4 rows instead of one [1, S] copy per 2 rows.
"""

import os
from contextlib import ExitStack

import numpy as np

B, S, E2, E = 2, 512, 512, 256
N_CORES = 8
ROWS = (B * S) // N_CORES
P = 128
HC = E2 // P
TC = S // P
DC = 2

_STATE = {}


def _build_nc(skew=3, pz_bufs=3):
    import concourse.bacc as bacc
    import concourse.bass as bass
    import concourse.tile as tile
    from concourse import mybir
    from concourse.masks import make_identity

    f32 = mybir.dt.float32
    f32r = mybir.dt.float32r
    bf16 = mybir.dt.bfloat16

    nc = bacc.Bacc("TRN2", target_bir_lowering=False, debug=False, num_devices=8)
    qt_h = nc.dram_tensor("qt", [E2, S], bf16, kind="ExternalInput")
    v_h = nc.dram_tensor("v", [S, E2], bf16, kind="ExternalInput")
    w_h = nc.dram_tensor("w", [E2, E], bf16, kind="ExternalInput")
    vd2_h = nc.dram_tensor("vd2", [2 * E], bf16, kind="ExternalInput")
    ctx_h = nc.dram_tensor("ctx_out", [ROWS, E2], f32, kind="ExternalOutput")
    att_h = nc.dram_tensor("att_out", [ROWS, S], f32, kind="ExternalOutput")
    qt, v, w = qt_h.ap(), v_h.ap(), w_h.ap()
    ctx_out, att_out = ctx_h.ap(), att_h.ap()

    with tile.TileContext(nc) as tc, ExitStack() as ctx:
        consts = ctx.enter_context(tc.tile_pool(name="consts", bufs=1))
        w2_pool = ctx.enter_context(tc.tile_pool(name="w2", bufs=6))
        zt_pool = ctx.enter_context(tc.tile_pool(name="zt", bufs=10))
        ps_pool = ctx.enter_context(tc.tile_pool(name="ps", bufs=1, space="PSUM"))
        stage_pool = ctx.enter_context(tc.tile_pool(name="stage", bufs=2))
        tail_pool = ctx.enter_context(tc.tile_pool(name="tail", bufs=1))

        qt_bf = consts.tile([P, HC, S], bf16)
        w_bf = consts.tile([P, HC, E], bf16)
        for hc in range(HC):
            # interleave qt/w chunk loads: the first w2 build + matmuls
            # need only (qt0, w0), so they start after ~2 small DMAs
            nc.sync.dma_start(out=qt_bf[:, hc, :], in_=qt[hc*P:(hc+1)*P, :])
            nc.sync.dma_start(out=w_bf[:, hc, :], in_=w[hc*P:(hc+1)*P, :])
        vd_sb = consts.tile([P, DC, 1], bf16)
        vd_src = bass.AP(tensor=vd2_h, offset=0, ap=[[1, P], [P, DC]])
        nc.sync.dma_start(out=vd_sb[:, :, 0], in_=vd_src)
        ident = consts.tile([P, P], bf16)
        make_identity(nc, ident[:])
        ident_f = consts.tile([P, P], f32)
        make_identity(nc, ident_f[:])

        qt_bf_base = qt_bf[:]

        def emit_front(j):
            w2 = w2_pool.tile([P, HC, E], bf16, tag="w2", name=f"w2_{j}")
            if j < 4:
                # startup: per-hc build so hc-0 matmuls start after the
                # first qt chunk lands instead of all four
                for hc in range(HC):
                    qb = bass.AP(tensor=qt_bf_base.tensor,
                                 offset=qt_bf_base.offset + hc * S + j,
                                 ap=[[qt_bf_base.ap[0][0], P], [0, E]])
                    nc.vector.tensor_tensor(out=w2[:, hc, :],
                                            in0=w_bf[:, hc, :], in1=qb,
                                            op=mybir.AluOpType.mult)
            else:
                qb = bass.AP(tensor=qt_bf_base.tensor,
                             offset=qt_bf_base.offset + j,
                             ap=[[qt_bf_base.ap[0][0], P], [S, HC], [0, E]])
                nc.vector.tensor_tensor(out=w2[:], in0=w_bf[:], in1=qb,
                                        op=mybir.AluOpType.mult)
            pz = ps_pool.tile([P, DC, S], f32, tag="pz", bufs=pz_bufs,
                              name=f"pz_{j}")
            for dc in range(DC):
                for hc in range(HC):
                    nc.tensor.matmul(pz[:, dc, j:S],
                                     lhsT=w2[:, hc, dc*P:(dc+1)*P],
                                     rhs=qt_bf[:, hc, j:S],
                                     start=(hc == 0), stop=(hc == HC-1))
            zt = zt_pool.tile([P, DC, S], bf16, tag="zt", name=f"zt_{j}")
            nc.scalar.activation(zt[:, :, j:S], pz[:, :, j:S],
                                 mybir.ActivationFunctionType.Tanh)
            return zt

        def emit_reduce_group(j0, zts):
            # 4 rows in 4 PE column strips; scores land on psum
            # partitions {0, 32, 64, 96} of one bank
            psr = ps_pool.tile([P, S], f32, tag="psr", bufs=2,
                               name=f"psr_{j0}")
            for u in range(4):
                j = j0 + u
                zt = zts.pop(j)
                for dc in range(DC):
                    nc.tensor.matmul(psr[32*u:32*u+1, j:S],
                                     lhsT=vd_sb[:, dc, :],
                                     rhs=zt[:, dc, j:S],
                                     start=(dc == 0), stop=(dc == DC-1),
                                     tile_position=(0, 32*u))
            stage = stage_pool.tile([P, S], f32, tag="rowstage",
                                    name=f"rowstage_{j0}")
            # full-partition copy costs the same per lane as a single row;
            # rows sit at partitions {0,32,64,96}, the rest is junk.
            # On vector, not scalar: the scalar FIFO must stay free for
            # tanh, which gates pz-buffer recycling.
            nc.vector.tensor_copy(stage[:, j0:S], psr[:, j0:S])
            # partition-crossing SBUF->SBUF DMA straight into the softmax
            # tile -- no DRAM round trip. Issue from different engine
            # queues so the last group's four issues don't serialize.
            qeng = [nc.sync, nc.gpsimd, nc.scalar, nc.gpsimd]
            for u in range(4):
                j = j0 + u
                qeng[u].dma_start(out=sjt_gather[j:j+1, j:S],
                                  in_=stage[32*u:32*u+1, j:S])

        sjt_gather = tail_pool.tile([P, S], f32)
        zts = {}
        for step in range(ROWS + skew + 4):
            if step < ROWS:
                zts[step] = emit_front(step)
            r = step - skew
            if r >= 0 and r % 4 == 3 and r - 3 < ROWS:
                emit_reduce_group(r - 3, zts)
            if step == 1:
                # v only needed at the tail — load behind the first rows
                v_sb = consts.tile([P, TC, E2], bf16)
                for tcc in range(TC):
                    nc.sync.dma_start(out=v_sb[:, tcc, :],
                                      in_=v[tcc*P:(tcc+1)*P, :])

        # tail: fix the diagonal block, softmax + context
        sjt = sjt_gather
        # reconstruct lower triangle of the own-diagonal block:
        #   sjt[a, b] (a > b) = sjt[b, a]
        # keep upper-incl-diag of the computed block (y >= x), else 0.
        # Out-of-place into its own tile: runs concurrent with the PE
        # transpose instead of serializing behind it on the RAW chain.
        sjt_u = tail_pool.tile([P, P], f32)
        nc.gpsimd.affine_select(out=sjt_u[:], in_=sjt[:, 0:P],
                                compare_op=mybir.AluOpType.is_ge, fill=0.0,
                                base=0, pattern=[[1, P]],
                                channel_multiplier=-1)
        pT = ps_pool.tile([P, P], f32, tag="pz", bufs=pz_bufs, name="pT")
        nc.tensor.transpose(pT[:], sjt[:, 0:P], ident_f[:])
        tT = tail_pool.tile([P, P], f32)
        nc.vector.tensor_copy(tT[:], pT[:])
        # keep strict-lower of the transpose (x > y), else 0
        nc.gpsimd.affine_select(out=tT[:], in_=tT[:],
                                compare_op=mybir.AluOpType.is_ge, fill=0.0,
                                base=-1, pattern=[[-1, P]],
                                channel_multiplier=1)
        nc.vector.tensor_tensor(out=sjt[:, 0:P], in0=sjt_u[:], in1=tT[:],
                                op=mybir.AluOpType.add)

        # no max-subtraction: |sjt| <= sum|vd| ~ 13, exp is fp32-safe and
        # softmax is shift-invariant. The non-diagonal columns don't need
        # the diag fix, so their exp/cast/transposes overlap it.
        att = tail_pool.tile([P, S], f32)
        denom_a = tail_pool.tile([P, 1], f32)
        denom_b = tail_pool.tile([P, 1], f32)
        att_bf = tail_pool.tile([P, S], bf16)
        nc.scalar.activation(att[:, P:S], sjt[:, P:S],
                             mybir.ActivationFunctionType.Exp,
                             bias=0.0, scale=1.0, accum_out=denom_b[:])
        nc.vector.tensor_copy(att_bf[:, P:S], att[:, P:S])
        nc.scalar.activation(att[:, 0:P], sjt[:, 0:P],
                             mybir.ActivationFunctionType.Exp,
                             bias=0.0, scale=1.0, accum_out=denom_a[:])
        nc.vector.tensor_copy(att_bf[:, 0:P], att[:, 0:P])
        denom = tail_pool.tile([P, 1], f32)
        nc.vector.tensor_tensor(out=denom[:], in0=denom_a[:], in1=denom_b[:],
                                op=mybir.AluOpType.add)
        rdenom = tail_pool.tile([P, 1], f32)
        nc.vector.reciprocal(rdenom[:], denom[:])
        atten = tail_pool.tile([P, S], f32)
        nc.vector.tensor_scalar_mul(atten[:], att[:], rdenom[:])
        nc.sync.dma_start(out=att_out[:, :], in_=atten[:])

        # context from UNNORMALIZED exp; scale rows by 1/denom afterwards.
        # Transpose non-diag blocks first: their operand is ready earlier,
        # and psum accumulation order is free (start on the first emitted).
        attT = tail_pool.tile([P, TC, P], bf16)
        order = [1, 2, 3, 0]
        for tcc in order:
            pt2 = ps_pool.tile([P, P], bf16, tag="pz", bufs=pz_bufs,
                               name=f"pt2_{tcc}")
            nc.tensor.transpose(pt2[:], att_bf[:, tcc*P:(tcc+1)*P], ident[:])
            nc.vector.tensor_copy(attT[:, tcc, :], pt2[:])
        pc = ps_pool.tile([P, E2], f32, tag="psr", bufs=2, name="pc")
        for k, tcc in enumerate(order):
            nc.tensor.matmul(pc[:], lhsT=attT[:, tcc, :], rhs=v_sb[:, tcc, :],
                             start=(k == 0), stop=(k == TC-1))
        ctx_sb = tail_pool.tile([P, E2], f32)
        nc.scalar.activation(ctx_sb[:], pc[:],
                             mybir.ActivationFunctionType.Identity,
                             scale=rdenom[:])
        nc.sync.dma_start(out=ctx_out[:, :], in_=ctx_sb[:])

    nc.compile()
    return nc


def _get_nc():
    if "nc" not in _STATE:
        _STATE["nc"] = _build_nc()
    return _STATE["nc"]


def kernel(query, value, Wd, vd):
    import ml_dtypes
    from concourse.bass_utils import run_bass_kernel_spmd

    bf = ml_dtypes.bfloat16
    query = np.asarray(query, dtype=np.float32).astype(bf)
    value = np.asarray(value, dtype=np.float32).astype(bf)
    Wd = np.asarray(Wd, dtype=np.float32).astype(bf)
    vd = np.asarray(vd, dtype=np.float32).astype(bf)

    vd2 = np.concatenate([vd, vd])
    in_maps = []
    for c in range(N_CORES):
        b, s0 = divmod(c * ROWS, S)
        qt = np.ascontiguousarray(np.roll(query[b].T, -s0, axis=1))
        vr = np.ascontiguousarray(np.roll(value[b], -s0, axis=0))
        in_maps.append({"qt": qt, "v": vr, "w": Wd, "vd2": vd2})

    nc = _get_nc()
    trace = bool(int(os.environ.get("BASS_KERNEL_TRACE", "0")))
    res = run_bass_kernel_spmd(nc, in_maps, list(range(N_CORES)), trace=trace)
    _STATE["last_result"] = res

    context = np.empty((B, S, E2), np.float32)
    atten = np.empty((B, S, S), np.float32)
    for c in range(N_CORES):
        b, s0 = divmod(c * ROWS, S)
        context[b, s0:s0 + ROWS] = res.results[c]["ctx_out"]
        atten[b, s0:s0 + ROWS] = np.roll(res.results[c]["att_out"], s0, axis=1)
    return context, atten
